# revision 33
# baseline (speedup 1.0000x reference)
"""Multi-head attention block on 8 Trainium2 NeuronCores.

Sharding: batch (B=2) x head-groups (4 heads each) -> 8 cores.
Each core computes q/k/v projections for its 4 heads of its batch,
causal attention, and a partial output projection; the host sums the 4
partials per batch and adds the bias.

Layout: projections produce qT/kT ([head_dim, seq]) and v in natural
per-chunk layout ([seq-chunk, head_dim] with a ones column per head for
the softmax denominator).  Scores are computed transposed (S^T [k, q])
so exp/masking work on [k, q] tiles, then the PV matmul flips back to
natural A [q, head_dim(+denom)] with the q tokens on the partition axis
(m=128, n=65/head: half the moving-dim cycles of the [65, q] form).
The denominator lands per-partition, so normalization is a cheap
per-partition tensor_scalar multiply; the normalized A is transposed
back to [head_dim, q] for the output projection with the DMA XBAR
transpose (off the PE entirely).
Biases: b_q/b_k applied on-chip (per-partition); b_v and b_out folded
into a host-side constant (b_out + w_out @ b_v) added after gathering.
"""

import os
import re
import sys

sys.path.insert(0, "/opt/trn_rl_repo")

import numpy as np
import ml_dtypes

import concourse.bass as bass
import concourse.mybir as mybir
import concourse.tile as tile

BF16 = mybir.dt.bfloat16
F32 = mybir.dt.float32
BF16_NP = ml_dtypes.bfloat16

N_CORES = 8
B = 2
S = 2048
D_MODEL = 1024
H_TOTAL = 16
D_K = 64
H_PER_CORE = 4                      # heads per core
HD = H_PER_CORE * D_K               # 256 head-dims per core
CORES_PER_BATCH = N_CORES // B

QB = 512                            # q-block (matmul moving free dim)
KC = 128                            # k-chunk (contraction tile)

N_PROCS = 27


def _split_waits_json(bir_json: bytes, limit: int = 1) -> bytes:
    """walrus in this env rejects >limit sync-waits on an instruction.
    Hoist excess waits onto fresh NoOps inserted just before, on the same
    engine queue (queue execution is serial, so ordering is identical)."""
    import orjson

    m = orjson.loads(bir_json)
    ctr = 0
    for fn in m.get("functions", []):
        for bb in fn.get("blocks", []):
            insts = bb.get("instructions") or []
            if not any(
                len((i.get("sync_info") or {}).get("on_wait") or []) > limit
                for i in insts
            ):
                continue
            out = []
            for inst in insts:
                si = inst.get("sync_info")
                waits = (si or {}).get("on_wait") or []
                if len(waits) > limit:
                    for w in waits[:-limit]:
                        ctr += 1
                        out.append(
                            {
                                "debug": inst.get("debug", 0),
                                "engine": inst["engine"],
                                "ins": [],
                                "outs": [],
                                "name": f"WSPL-{ctr}",
                                "opcode": "NoOp",
                                "sync_info": {"on_update": [], "on_wait": [w]},
                            }
                        )
                    si["on_wait"] = waits[-limit:]
                out.append(inst)
            bb["instructions"] = out
    return orjson.dumps(m)


LAST_PREDICTED_NS = None


def _install_schedule_capture():
    """Record the Tile scheduler's cost-model makespan for each build."""
    if getattr(tile.TileContext, "_capture_installed", False):
        return
    orig = tile.TileContext.schedule_block

    def wrapped(self, *a, **kw):
        r = orig(self, *a, **kw)
        try:
            global LAST_PREDICTED_NS
            LAST_PREDICTED_NS = r[1].time
        except Exception:
            pass
        return r

    tile.TileContext.schedule_block = wrapped
    tile.TileContext._capture_installed = True


def _install_compile_patch():
    import concourse.bass_utils as bu
    import concourse.bass2jax as b2j

    if getattr(bu, "_wait_split_installed", False):
        return
    orig = bu.compile_bir_kernel

    def wrapped(bir_json, tmpdir, neff_name="file.neff"):
        return orig(_split_waits_json(bytes(bir_json)), tmpdir, neff_name)

    bu.compile_bir_kernel = wrapped
    b2j.compile_bir_kernel = wrapped
    bu._wait_split_installed = True


def build_program(mask_mode="causal", s=S, d=D_MODEL, heads=H_PER_CORE,
                  epool_bufs=34, opool_bufs=3):
    """One SPMD program; per-core behavior differs only via inputs.

    mask_mode: "causal" (skip above-diagonal chunks, affine-select the
    diagonal ones), "ones" (no masking), "general" (multiplicative 0/1
    mask loaded from DRAM, pre-transposed host-side).
    """
    _install_compile_patch()
    _install_schedule_capture()
    hd = heads * D_K
    nq = s // QB          # q blocks
    nkc = s // KC         # k chunks
    nqc = QB // KC        # q chunks per q block
    dch = d // 128        # contraction chunks for projections
    npair = heads // 2    # head pairs (even head on partitions 0-63)
    assert hd % 128 == 0 and hd // 128 == npair

    nc = bass.Bass()
    xq = nc.dram_tensor("xq", [d, s], BF16, kind="ExternalInput")
    xk = nc.dram_tensor("xk", [d, s], BF16, kind="ExternalInput")
    xv = nc.dram_tensor("xv", [d, s], BF16, kind="ExternalInput")
    # weights arrive pre-packed host-side so each partition's line is one
    # contiguous 4KB run: wq[p, c*hd+m] = w_q.T[c*128+p, m] etc.
    wq = nc.dram_tensor("wq", [128, dch * hd], BF16, kind="ExternalInput")
    wk = nc.dram_tensor("wk", [128, dch * hd], BF16, kind="ExternalInput")
    wv = nc.dram_tensor("wv", [128, dch * hd], BF16, kind="ExternalInput")
    wo = nc.dram_tensor("wo", [128, npair * d], BF16, kind="ExternalInput")
    bq = nc.dram_tensor("bq", [128, npair], F32, kind="ExternalInput")
    bk = nc.dram_tensor("bk", [128, npair], F32, kind="ExternalInput")
    if mask_mode == "general":
        m01 = nc.dram_tensor("m01", [s, s], BF16, kind="ExternalInput")
    out = nc.dram_tensor("out", [s, d], BF16, kind="ExternalOutput")

    xq_r = xq[:, :].rearrange("(c p) s -> p c s", p=128)
    xk_r = xk[:, :].rearrange("(c p) s -> p c s", p=128)
    xv_r = xv[:, :].rearrange("(c p) s -> p c s", p=128)
    wq_r = wq[:, :].rearrange("p (c m) -> p c m", m=hd)
    wk_r = wk[:, :].rearrange("p (c m) -> p c m", m=hd)
    wv_r = wv[:, :].rearrange("p (c m) -> p c m", m=hd)
    wo_r = wo[:, :].rearrange("p (c e) -> p c e", e=d)

    with tile.TileContext(nc) as tc:
        with (
            tc.tile_pool(name="consts", bufs=1) as consts,
            tc.tile_pool(name="qkres", bufs=1) as qkres,
            tc.tile_pool(name="xkp", bufs=2) as xkp,
            tc.tile_pool(name="xqp", bufs=2) as xqp,
            tc.tile_pool(name="xvp", bufs=2) as xvp,
            tc.tile_pool(name="epool", bufs=epool_bufs) as epool,
            tc.tile_pool(name="anp", bufs=4) as anp,
            tc.tile_pool(name="atp", bufs=2) as atp,
            tc.tile_pool(name="opool", bufs=opool_bufs) as opool,
            tc.tile_pool(name="rpool", bufs=4) as rpool,
            tc.tile_pool(name="mpool", bufs=16) as mpool,
            tc.tile_pool(name="pp", bufs=2, space="PSUM") as pp,
            tc.tile_pool(name="sp", bufs=2, space="PSUM") as sp,
            tc.tile_pool(name="ap2", bufs=2, space="PSUM") as ap2,
        ):
            # --- weights + biases on the Activation DMA queue; x streams
            # on SP so both queues issue in parallel at startup.  wk is
            # split so the very first projection matmul only waits for a
            # small transfer.
            wk_sb = consts.tile([128, dch, hd], BF16, tag="wk")
            nc.sync.dma_start(wk_sb[:, 0:2, :], wk_r[:, 0:2, :])
            # PE warm-up: dummy matmuls ramp the tensor engine out of its
            # low p-state while the first input DMAs are still in flight.
            warm = consts.tile([1, QB], BF16, tag="warm")
            nc.vector.memset(warm, 0.0)
            wps = sp.tile([128, 2 * QB], F32, tag="sp", name="warmps")
            for _ in range(6):
                nc.tensor.matmul(
                    wps[:, 0:QB], lhsT=warm[:, 0:128], rhs=warm,
                    start=True, stop=True,
                )
            nc.sync.dma_start(wk_sb[:, 2:dch, :], wk_r[:, 2:dch, :])
            bk_sb = consts.tile([128, npair], F32, tag="bk")
            nc.scalar.dma_start(bk_sb, bk[:, :])
            bq_sb = consts.tile([128, npair], F32, tag="bq")
            nc.scalar.dma_start(bq_sb, bq[:, :])
            wq_sb = consts.tile([128, dch, hd], BF16, tag="wq")
            nc.scalar.dma_start(wq_sb, wq_r)
            wv_sb = consts.tile([128, dch, hd], BF16, tag="wv")
            nc.scalar.dma_start(wv_sb, wv_r)
            wo_sb = consts.tile([128, npair, d], BF16, tag="wo")
            nc.scalar.dma_start(wo_sb, wo_r)

            # persistent per-core tensors
            qT = qkres.tile([128, npair, s], BF16, tag="qT")
            kT = qkres.tile([128, npair, s], BF16, tag="kT")
            v_sb = qkres.tile([128, nkc, heads * 65], BF16, tag="v")
            # fill with 1.0 once; v-proj copies overwrite cols 0:64 of each
            # 65-block, leaving column 64 = 1.0 (softmax denominator trick)
            nc.vector.memset(v_sb, 1.0)

            def load_x(qb, fine):
                """Fetch the qb-th quarter of xk/xq/xv into rotating tiles.
                fine=True splits into 2-dc pieces so the first projections
                can start before the whole quarter has landed."""
                s_lo = qb * QB
                tiles = {}
                order = (
                    ((xkp, xk_r, "xk"), (xqp, xq_r, "xq"), (xvp, xv_r, "xv"))
                    if fine
                    else ((xqp, xq_r, "xq"), (xkp, xk_r, "xk"), (xvp, xv_r, "xv"))
                )
                for pool, dram, tg in order:
                    t = pool.tile([128, dch, QB], BF16, tag=tg)
                    if fine:
                        for i2 in range(dch // 2):
                            nc.sync.dma_start(
                                t[:, 2 * i2 : 2 * i2 + 2, :],
                                dram[:, 2 * i2 : 2 * i2 + 2, s_lo : s_lo + QB],
                            )
                    else:
                        nc.sync.dma_start(t, dram[:, :, s_lo : s_lo + QB])
                    tiles[tg] = t
                return tiles["xk"], tiles["xq"], tiles["xv"]

            def proj_block(qb, x_tiles):
                s_lo = qb * QB
                xk_t, xq_t, xv_t = x_tiles

                def kq_proj_group(x_t, w_sb, b_sb, dst, c2):
                    ps = pp.tile([128, QB], F32, tag="pp", name="ps")
                    for dc in range(dch):
                        nc.tensor.matmul(
                            ps,
                            lhsT=w_sb[:, dc, c2 * 128 : (c2 + 1) * 128],
                            rhs=x_t[:, dc, 0:QB],
                            start=(dc == 0),
                            stop=(dc == dch - 1),
                        )
                    nc.vector.tensor_scalar_add(
                        dst[:, c2, s_lo : s_lo + QB], ps, b_sb[:, c2 : c2 + 1]
                    )

                def v_proj_group(sc):
                    sck = qb * nqc + sc
                    ps = pp.tile([128, hd], F32, tag="pp", name="ps")
                    for dc in range(dch):
                        nc.tensor.matmul(
                            ps,
                            lhsT=xv_t[:, dc, sc * 128 : (sc + 1) * 128],
                            rhs=wv_sb[:, dc, :],
                            start=(dc == 0),
                            stop=(dc == dch - 1),
                        )
                    nc.vector.tensor_copy(
                        v_sb[:, sck, :].rearrange("p (h j) -> p h j", j=65)[:, :, 0:64],
                        ps[:].rearrange("p (h j) -> p h j", j=64),
                    )

                return kq_proj_group, v_proj_group, xk_t, xq_t

            def proj_q(groups):
                kq_proj_group, _, _, xq_t = groups
                for c2 in range(npair):
                    kq_proj_group(xq_t, wq_sb, bq_sb, qT, c2)

            def proj_k(groups):
                kq_proj_group, _, xk_t, _ = groups
                for c2 in range(npair):
                    kq_proj_group(xk_t, wk_sb, bk_sb, kT, c2)

            def proj_v(groups):
                _, v_proj_group, _, _ = groups
                for sc in range(nqc):
                    v_proj_group(sc)

            def attn_block(qb, a_sb, mid_emit=None, tail_emit=None,
                           load_emit=None, borrow_pp=False):
                s_lo = qb * QB
                n_chunks = (qb + 1) * nqc if mask_mode == "causal" else nkc
                diag_lo = qb * nqc
                assert n_chunks % 2 == 0
                if mask_mode == "general":
                    m_tiles = []
                    for kc_i in range(n_chunks):
                        mt = mpool.tile([128, QB], BF16, tag="m")
                        nc.sync.dma_start(
                            mt, m01[kc_i * KC : (kc_i + 1) * KC, s_lo : s_lo + QB]
                        )
                        m_tiles.append(mt)

                def pv(a_ps, qci, kcp, e_cp):
                    """accumulate A[qci] over the two chunks of pair kcp.
                    start/stop stay False: a PSUM start flag zeroes the
                    whole 2KB bank, so the four per-head chains in one bank
                    are bracketed by full-width zero/stop matmuls instead
                    (see a_zero/a_stop) whose region overlap also pins the
                    scheduler ordering."""
                    qcg = diag_lo + qci
                    for ck in range(2):
                        kc_i = kcp + ck
                        if mask_mode == "causal" and kc_i > qcg:
                            continue
                        for h in range(heads):
                            nc.tensor.matmul(
                                a_ps[:, h * 65 : (h + 1) * 65],
                                lhsT=e_cp[h][:, ck * QB + qci * KC : ck * QB + (qci + 1) * KC],
                                rhs=v_sb[:, kc_i, h * 65 : (h + 1) * 65],
                                start=False,
                                stop=False,
                            )

                def a_zero(a_ps):
                    # open the bank's accumulation group: zeroes the bank,
                    # adds 0 over the full used range
                    nc.tensor.matmul(
                        a_ps, lhsT=warm[:, 0:128], rhs=warm[:, 0 : heads * 65],
                        start=True, stop=False,
                    )

                def a_stop(a_ps):
                    # close the group (adds 0); norm reads depend on this
                    nc.tensor.matmul(
                        a_ps, lhsT=warm[:, 0:128], rhs=warm[:, 0 : heads * 65],
                        start=False, stop=True,
                    )

                e_store = []
                if load_emit is not None:
                    # next block's x fetch issues now so the transfers hide
                    # under this sweep instead of gating the next q-proj
                    load_emit()
                mid_done = mid_emit is None
                # borrow_pp=True: the second half's accumulators come from
                # the pp pool (requested after this block's k/v projection
                # groups, so the pp FIFO stays deadlock-free).  Legal only
                # when no LATER projections will need pp.
                for half, qcis in enumerate(((0, 1), (2, 3))):
                    a_ps = {}
                    for qci in qcis:
                        pool = pp if (half == 1 and borrow_pp) else ap2
                        tg = "pp" if (half == 1 and borrow_pp) else "A"
                        a_ps[qci] = pool.tile(
                            [128, heads * 65], F32, tag=tg, name=f"aps{qci}"
                        )
                        a_zero(a_ps[qci])
                    for cp_i, kcp in enumerate(range(0, n_chunks, 2)):
                        if half == 0:
                            # emit this block's k/v projections just before
                            # the first diagonal chunk-pair needs them: the
                            # pp-pool FIFO and scheduler priorities then
                            # match the time order of needs
                            if not mid_done and kcp + 1 >= diag_lo:
                                mid_emit()
                                mid_done = True
                            # --- scores + exp for both head pairs; k-chunk
                            # PAIRS share one [128, 2*QB] psum tile (2 banks)
                            # so exp runs as a single wide ACT op ---
                            if mask_mode == "causal":
                                skips = [
                                    max(0, ((kcp + ck) - diag_lo) * KC)
                                    if (kcp + ck) >= diag_lo
                                    else 0
                                    for ck in range(2)
                                ]
                            else:
                                skips = [0, 0]
                            e_cp = [None] * heads
                            for pr in range(npair):
                                s_pss = [
                                    sp.tile([128, 2 * QB], F32, tag="sp", name=f"sps{sub}")
                                    for sub in range(2)
                                ]
                                # emit the four score matmuls alternating
                                # head-subs: adjacent MMs then occupy disjoint
                                # PE row-groups (partitions 0-63 vs 64-127)
                                for ck in range(2):
                                    sk = skips[ck]
                                    for sub in range(2):
                                        rows = slice(sub * 64, sub * 64 + 64)
                                        nc.tensor.matmul(
                                            s_pss[sub][:, ck * QB + sk : (ck + 1) * QB],
                                            lhsT=kT[rows, pr, (kcp + ck) * KC : (kcp + ck + 1) * KC],
                                            rhs=qT[rows, pr, s_lo + sk : s_lo + QB],
                                            start=True,
                                            stop=True,
                                        )
                                for sub in range(2):
                                    s_ps = s_pss[sub]
                                    e = epool.tile([128, 2 * QB], BF16, tag="e")
                                    if skips[0] == 0 and skips[1] == 0:
                                        nc.scalar.activation(
                                            out=e, in_=s_ps,
                                            func=mybir.ActivationFunctionType.Exp,
                                        )
                                    else:
                                        for ck in range(2):
                                            sk = skips[ck]
                                            nc.scalar.activation(
                                                out=e[:, ck * QB + sk : (ck + 1) * QB],
                                                in_=s_ps[:, ck * QB + sk : (ck + 1) * QB],
                                                func=mybir.ActivationFunctionType.Exp,
                                            )
                                    for ck in range(2):
                                        kc_i = kcp + ck
                                        sk = skips[ck]
                                        if mask_mode == "causal" and kc_i >= diag_lo:
                                            # keep condition is (f' - p) >= 0
                                            nc.gpsimd.affine_select(
                                                out=e[:, ck * QB + sk : (ck + 1) * QB],
                                                in_=e[:, ck * QB + sk : (ck + 1) * QB],
                                                compare_op=mybir.AluOpType.is_ge,
                                                fill=0.0,
                                                base=0,
                                                pattern=[[1, QB - sk]],
                                                channel_multiplier=-1,
                                            )
                                        if mask_mode == "general":
                                            nc.vector.tensor_mul(
                                                e[:, ck * QB : (ck + 1) * QB],
                                                e[:, ck * QB : (ck + 1) * QB],
                                                m_tiles[kc_i],
                                            )
                                    e_cp[pr * 2 + sub] = e
                            e_store.append(e_cp)
                        for qci in qcis:
                            pv(a_ps[qci], qci, kcp, e_store[cp_i])
                    if half == 0 and not mid_done:
                        mid_emit()
                        mid_done = True
                    # --- normalize + transpose each finished q chunk ---
                    for qci in qcis:
                        aps = a_ps[qci]
                        a_stop(aps)
                        aps_r = aps[:].rearrange("p (h j) -> p h j", j=65)
                        rec = rpool.tile([128, heads, 1], F32, tag="r")
                        nc.vector.reciprocal(rec, aps_r[:, :, 64:65])
                        a_n = anp.tile([128, hd], BF16, tag="an")
                        for h in range(heads):
                            nc.vector.tensor_scalar_mul(
                                a_n[:, h * 64 : (h + 1) * 64],
                                aps_r[:, h, 0:64],
                                rec[:, h, :],
                            )
                        for pr in range(npair):
                            nc.sync.dma_start(
                                a_sb[:, pr, qci * KC : (qci + 1) * KC],
                                a_n[:, pr * 128 : (pr + 1) * 128],
                                transpose=True,
                            )
                    if half == 0 and tail_emit is not None:
                        # next block's q projection (and x fetch) outranks
                        # this block's second-half PV replay: it is the
                        # critical path of the next sweep
                        tail_emit()

            def out_proj(qb, a_sb, tail=False):
                s_lo = qb * QB
                for qcr in range(nqc):
                    o_sb = opool.tile([128, d], BF16, tag="o")
                    for nb in range(d // QB):
                        # ap2 (not sp): during sweeps the scores own sp, so
                        # out-proj would starve until the sweep boundary;
                        # ap2 slots free right after the norms instead.
                        ops = ap2.tile([128, QB], F32, tag="A", name="ops")
                        for c2 in range(npair):
                            nc.tensor.matmul(
                                ops,
                                lhsT=a_sb[:, c2, qcr * KC : (qcr + 1) * KC],
                                rhs=wo_sb[:, c2, nb * QB : (nb + 1) * QB],
                                start=(c2 == 0),
                                stop=(c2 == npair - 1),
                            )
                        # DVE (the Pool engine has no PSUM port); in the
                        # tail the Act engine is idle, split with it there
                        if tail and nb % 2 == 1:
                            nc.scalar.copy(
                                o_sb[:, nb * QB : (nb + 1) * QB], ops
                            )
                        else:
                            nc.vector.tensor_copy(
                                o_sb[:, nb * QB : (nb + 1) * QB], ops
                            )
                    nc.sync.dma_start(
                        out[(s_lo + qcr * KC) : (s_lo + (qcr + 1) * KC), :], o_sb
                    )

            if mask_mode == "causal":
                # attention(qb) only reads k/v ranges projected so far.
                # Emission order sets scheduler priority AND the FIFO grant
                # order of the psum pools, so it must match the time order
                # of needs: q(qb) -> sweep(qb) non-diag -> k/v(qb) ->
                # sweep(qb) diag -> q(qb+1) -> PV replay -> out-proj(qb).
                groups_of = {}

                def make_load(nqb):
                    def load():
                        groups_of[nqb] = proj_block(nqb, load_x(nqb, fine=False))
                    return load

                def make_tail(nqb):
                    def tail():
                        proj_q(groups_of[nqb])
                    return tail

                def make_mid(qb):
                    def mid():
                        proj_k(groups_of[qb])
                        proj_v(groups_of[qb])
                    return mid

                groups_of[0] = proj_block(0, load_x(0, fine=True))
                proj_k(groups_of[0])
                proj_q(groups_of[0])
                proj_v(groups_of[0])
                for qb in range(nq):
                    a_sb = atp.tile([128, npair, QB], BF16, tag="aT")
                    attn_block(
                        qb,
                        a_sb,
                        mid_emit=(make_mid(qb) if qb > 0 else None),
                        tail_emit=(make_tail(qb + 1) if qb + 1 < nq else None),
                        load_emit=(make_load(qb + 1) if qb + 1 < nq else None),
                        borrow_pp=(qb == nq - 1),
                    )
                    out_proj(qb, a_sb, tail=(qb == nq - 1))
            else:
                # unmasked attention reads ALL k/v: project everything first
                # (pp is idle during attention, so every block can borrow)
                for qb in range(nq):
                    groups = proj_block(qb, load_x(qb, fine=(qb == 0)))
                    proj_k(groups)
                    proj_q(groups)
                    proj_v(groups)
                for qb in range(nq):
                    a_sb = atp.tile([128, npair, QB], BF16, tag="aT")
                    attn_block(qb, a_sb, borrow_pp=True)
                    out_proj(qb, a_sb)

    return nc


# ---------------------------------------------------------------------------
# host side
# ---------------------------------------------------------------------------

_PROG_CACHE = {}
LAST_RESULT = None


def _get_program(mask_mode):
    if mask_mode not in _PROG_CACHE:
        _PROG_CACHE[mask_mode] = build_program(mask_mode)
    return _PROG_CACHE[mask_mode]


def _bf16(a):
    return np.ascontiguousarray(a).astype(BF16_NP)


def _pack_w(wT):
    """[D, m] -> [128, (D//128)*m] with row p holding chunks contiguously."""
    dch_, m = wT.shape[0] // 128, wT.shape[1]
    return np.ascontiguousarray(
        wT.reshape(dch_, 128, m).transpose(1, 0, 2).reshape(128, dch_ * m)
    )


def _pack_b(b):
    """[HD] -> [128, npair]: column c2 holds partitions of head-pair c2."""
    npair = HD // 128
    return np.ascontiguousarray(b.reshape(npair, 128).T).astype(np.float32)


def kernel(query, key_in, value, mask, w_q, b_q, w_k, b_k, w_v, b_v, w_out, b_out):
    from concourse.bass_utils import run_bass_kernel_spmd

    query = np.asarray(query, dtype=np.float32)
    key_in = np.asarray(key_in, dtype=np.float32)
    value = np.asarray(value, dtype=np.float32)
    mask = np.asarray(mask)
    w_q = np.asarray(w_q, dtype=np.float32)
    b_q = np.asarray(b_q, dtype=np.float32)
    w_k = np.asarray(w_k, dtype=np.float32)
    b_k = np.asarray(b_k, dtype=np.float32)
    w_v = np.asarray(w_v, dtype=np.float32)
    b_v = np.asarray(b_v, dtype=np.float32)
    w_out = np.asarray(w_out, dtype=np.float32)
    b_out = np.asarray(b_out, dtype=np.float32)

    scale = 1.0 / np.sqrt(np.float32(D_K))

    if (mask == 1).all():
        mode = "ones"
    elif all(
        np.array_equal(mask[b, 0], np.tril(np.ones((S, S), mask.dtype)))
        for b in range(mask.shape[0])
    ):
        mode = "causal"
    else:
        mode = "general"
    nc = _get_program(mode)

    wqT = _bf16(w_q.T * scale)   # [D, D] scaled
    wkT = _bf16(w_k.T)
    wvT = _bf16(w_v.T)
    woT = _bf16(w_out.T)
    bq_s = (b_q * scale).astype(np.float32)

    # per-batch transposed activations, shared by the 4 cores of a batch
    xqT = [_bf16(query[b].T) for b in range(B)]
    xkT = [_bf16(key_in[b].T) for b in range(B)]
    xvT = [_bf16(value[b].T) for b in range(B)]
    m01T = [_bf16(mask[b, 0].T) for b in range(B)] if mode == "general" else None

    in_maps = []
    for c in range(N_CORES):
        b = c // CORES_PER_BATCH
        hg = c % CORES_PER_BATCH
        hsl = slice(hg * HD, (hg + 1) * HD)
        im = {
            "xq": xqT[b],
            "xk": xkT[b],
            "xv": xvT[b],
            "wq": _pack_w(wqT[:, hsl]),
            "wk": _pack_w(wkT[:, hsl]),
            "wv": _pack_w(wvT[:, hsl]),
            "wo": _pack_w(woT[hsl, :]),
            "bq": _pack_b(bq_s[hsl]),
            "bk": _pack_b(b_k[hsl]),
        }
        if mode == "general":
            im["m01"] = m01T[b]
        in_maps.append(im)

    global LAST_RESULT
    try:
        res = run_bass_kernel_spmd(nc, in_maps, list(range(N_CORES)))
    except Exception:
        # transient NRT_EXEC_UNIT_UNRECOVERABLE wedges have been observed on
        # this fabric; a single retry has always cleared them
        import time as _time

        _time.sleep(3.0)
        res = run_bass_kernel_spmd(nc, in_maps, list(range(N_CORES)))
    LAST_RESULT = res

    b_eff = b_out + w_out @ b_v
    out = np.zeros((B, S, D_MODEL), dtype=np.float32)
    for c in range(N_CORES):
        out[c // CORES_PER_BATCH] += res.results[c]["out"].astype(np.float32)
    out += b_eff[None, None, :]
    return out


# revision 35
# speedup vs baseline: 1.0216x; 1.0216x over previous
"""Multi-head attention block on 8 Trainium2 NeuronCores.

Sharding: batch (B=2) x head-groups (4 heads each) -> 8 cores.
Each core computes q/k/v projections for its 4 heads of its batch,
causal attention, and a partial output projection; the host sums the 4
partials per batch and adds the bias.

Layout: projections produce qT/kT ([head_dim, seq]) and v in natural
per-chunk layout ([seq-chunk, head_dim] with a ones column per head for
the softmax denominator).  Scores are computed transposed (S^T [k, q])
so exp/masking work on [k, q] tiles, then the PV matmul flips back to
natural A [q, head_dim(+denom)] with the q tokens on the partition axis
(m=128, n=65/head: half the moving-dim cycles of the [65, q] form).
The denominator lands per-partition, so normalization is a cheap
per-partition tensor_scalar multiply; the normalized A is transposed
back to [head_dim, q] for the output projection with the DMA XBAR
transpose (off the PE entirely).
Biases: b_q/b_k applied on-chip (per-partition); b_v and b_out folded
into a host-side constant (b_out + w_out @ b_v) added after gathering.
"""

import os
import re
import sys

sys.path.insert(0, "/opt/trn_rl_repo")

import numpy as np
import ml_dtypes

import concourse.bass as bass
import concourse.mybir as mybir
import concourse.tile as tile

BF16 = mybir.dt.bfloat16
F32 = mybir.dt.float32
BF16_NP = ml_dtypes.bfloat16

N_CORES = 8
B = 2
S = 2048
D_MODEL = 1024
H_TOTAL = 16
D_K = 64
H_PER_CORE = 4                      # heads per core
HD = H_PER_CORE * D_K               # 256 head-dims per core
CORES_PER_BATCH = N_CORES // B

QB = 512                            # q-block (matmul moving free dim)
KC = 128                            # k-chunk (contraction tile)

N_PROCS = 27


def _split_waits_json(bir_json: bytes, limit: int = 1) -> bytes:
    """walrus in this env rejects >limit sync-waits on an instruction.
    Hoist excess waits onto fresh NoOps inserted just before, on the same
    engine queue (queue execution is serial, so ordering is identical)."""
    import orjson

    m = orjson.loads(bir_json)
    ctr = 0
    for fn in m.get("functions", []):
        for bb in fn.get("blocks", []):
            insts = bb.get("instructions") or []
            if not any(
                len((i.get("sync_info") or {}).get("on_wait") or []) > limit
                for i in insts
            ):
                continue
            out = []
            for inst in insts:
                si = inst.get("sync_info")
                waits = (si or {}).get("on_wait") or []
                if len(waits) > limit:
                    for w in waits[:-limit]:
                        ctr += 1
                        out.append(
                            {
                                "debug": inst.get("debug", 0),
                                "engine": inst["engine"],
                                "ins": [],
                                "outs": [],
                                "name": f"WSPL-{ctr}",
                                "opcode": "NoOp",
                                "sync_info": {"on_update": [], "on_wait": [w]},
                            }
                        )
                    si["on_wait"] = waits[-limit:]
                out.append(inst)
            bb["instructions"] = out
    return orjson.dumps(m)


LAST_PREDICTED_NS = None


def _install_schedule_capture():
    """Record the Tile scheduler's cost-model makespan for each build."""
    if getattr(tile.TileContext, "_capture_installed", False):
        return
    orig = tile.TileContext.schedule_block

    def wrapped(self, *a, **kw):
        r = orig(self, *a, **kw)
        try:
            global LAST_PREDICTED_NS
            LAST_PREDICTED_NS = r[1].time
        except Exception:
            pass
        return r

    tile.TileContext.schedule_block = wrapped
    tile.TileContext._capture_installed = True


def _install_compile_patch():
    import concourse.bass_utils as bu
    import concourse.bass2jax as b2j

    if getattr(bu, "_wait_split_installed", False):
        return
    orig = bu.compile_bir_kernel

    def wrapped(bir_json, tmpdir, neff_name="file.neff"):
        return orig(_split_waits_json(bytes(bir_json)), tmpdir, neff_name)

    bu.compile_bir_kernel = wrapped
    b2j.compile_bir_kernel = wrapped
    bu._wait_split_installed = True


def build_program(mask_mode="causal", s=S, d=D_MODEL, heads=H_PER_CORE,
                  epool_bufs=34, opool_bufs=3):
    """One SPMD program; per-core behavior differs only via inputs.

    mask_mode: "causal" (skip above-diagonal chunks, affine-select the
    diagonal ones), "ones" (no masking), "general" (multiplicative 0/1
    mask loaded from DRAM, pre-transposed host-side).
    """
    _install_compile_patch()
    _install_schedule_capture()
    hd = heads * D_K
    nq = s // QB          # q blocks
    nkc = s // KC         # k chunks
    nqc = QB // KC        # q chunks per q block
    dch = d // 128        # contraction chunks for projections
    npair = heads // 2    # head pairs (even head on partitions 0-63)
    assert hd % 128 == 0 and hd // 128 == npair

    nc = bass.Bass()
    xq = nc.dram_tensor("xq", [d, s], BF16, kind="ExternalInput")
    xk = nc.dram_tensor("xk", [d, s], BF16, kind="ExternalInput")
    xv = nc.dram_tensor("xv", [d, s], BF16, kind="ExternalInput")
    # weights arrive pre-packed host-side so each partition's line is one
    # contiguous 4KB run: wq[p, c*hd+m] = w_q.T[c*128+p, m] etc.
    wq = nc.dram_tensor("wq", [128, dch * hd], BF16, kind="ExternalInput")
    wk = nc.dram_tensor("wk", [128, dch * hd], BF16, kind="ExternalInput")
    wv = nc.dram_tensor("wv", [128, dch * hd], BF16, kind="ExternalInput")
    wo = nc.dram_tensor("wo", [128, npair * d], BF16, kind="ExternalInput")
    bq = nc.dram_tensor("bq", [128, npair], F32, kind="ExternalInput")
    bk = nc.dram_tensor("bk", [128, npair], F32, kind="ExternalInput")
    if mask_mode == "general":
        m01 = nc.dram_tensor("m01", [s, s], BF16, kind="ExternalInput")
    out = nc.dram_tensor("out", [s, d], BF16, kind="ExternalOutput")

    xq_r = xq[:, :].rearrange("(c p) s -> p c s", p=128)
    xk_r = xk[:, :].rearrange("(c p) s -> p c s", p=128)
    xv_r = xv[:, :].rearrange("(c p) s -> p c s", p=128)
    wq_r = wq[:, :].rearrange("p (c m) -> p c m", m=hd)
    wk_r = wk[:, :].rearrange("p (c m) -> p c m", m=hd)
    wv_r = wv[:, :].rearrange("p (c m) -> p c m", m=hd)
    wo_r = wo[:, :].rearrange("p (c e) -> p c e", e=d)

    with tile.TileContext(nc) as tc:
        with (
            tc.tile_pool(name="consts", bufs=1) as consts,
            tc.tile_pool(name="qkres", bufs=1) as qkres,
            tc.tile_pool(name="xkp", bufs=2) as xkp,
            tc.tile_pool(name="xqp", bufs=2) as xqp,
            tc.tile_pool(name="xvp", bufs=2) as xvp,
            tc.tile_pool(name="epool", bufs=epool_bufs) as epool,
            tc.tile_pool(name="anp", bufs=4) as anp,
            tc.tile_pool(name="atp", bufs=2) as atp,
            tc.tile_pool(name="opool", bufs=opool_bufs) as opool,
            tc.tile_pool(name="rpool", bufs=4) as rpool,
            tc.tile_pool(name="mpool", bufs=16) as mpool,
            tc.tile_pool(name="pp", bufs=2, space="PSUM") as pp,
            tc.tile_pool(name="sp", bufs=2, space="PSUM") as sp,
            tc.tile_pool(name="ap2", bufs=2, space="PSUM") as ap2,
        ):
            # --- weights + biases on the Activation DMA queue; x streams
            # on SP so both queues issue in parallel at startup.  wk is
            # split so the very first projection matmul only waits for a
            # small transfer.
            wk_sb = consts.tile([128, dch, hd], BF16, tag="wk")
            nc.sync.dma_start(wk_sb[:, 0:2, :], wk_r[:, 0:2, :])
            # PE warm-up: dummy matmuls ramp the tensor engine out of its
            # low p-state while the first input DMAs are still in flight.
            warm = consts.tile([1, QB], BF16, tag="warm")
            nc.vector.memset(warm, 0.0)
            wps = sp.tile([128, 2 * QB], F32, tag="sp", name="warmps")
            for _ in range(6):
                nc.tensor.matmul(
                    wps[:, 0:QB], lhsT=warm[:, 0:128], rhs=warm,
                    start=True, stop=True,
                )
            nc.sync.dma_start(wk_sb[:, 2:dch, :], wk_r[:, 2:dch, :])
            bk_sb = consts.tile([128, npair], F32, tag="bk")
            nc.scalar.dma_start(bk_sb, bk[:, :])
            bq_sb = consts.tile([128, npair], F32, tag="bq")
            nc.scalar.dma_start(bq_sb, bq[:, :])
            wq_sb = consts.tile([128, dch, hd], BF16, tag="wq")
            nc.scalar.dma_start(wq_sb, wq_r)
            wv_sb = consts.tile([128, dch, hd], BF16, tag="wv")
            nc.scalar.dma_start(wv_sb, wv_r)
            wo_sb = consts.tile([128, npair, d], BF16, tag="wo")
            nc.scalar.dma_start(wo_sb, wo_r)

            # persistent per-core tensors
            qT = qkres.tile([128, npair, s], BF16, tag="qT")
            kT = qkres.tile([128, npair, s], BF16, tag="kT")
            v_sb = qkres.tile([128, nkc, heads * 65], BF16, tag="v")
            # fill with 1.0 once; v-proj copies overwrite cols 0:64 of each
            # 65-block, leaving column 64 = 1.0 (softmax denominator trick)
            nc.vector.memset(v_sb, 1.0)

            def load_x(qb, fine):
                """Fetch the qb-th quarter of xk/xq/xv into rotating tiles.
                fine=True splits into 2-dc pieces so the first projections
                can start before the whole quarter has landed."""
                s_lo = qb * QB
                tiles = {}
                order = (
                    ((xkp, xk_r, "xk"), (xqp, xq_r, "xq"), (xvp, xv_r, "xv"))
                    if fine
                    else ((xqp, xq_r, "xq"), (xkp, xk_r, "xk"), (xvp, xv_r, "xv"))
                )
                for pool, dram, tg in order:
                    t = pool.tile([128, dch, QB], BF16, tag=tg)
                    if fine:
                        for i2 in range(dch // 2):
                            nc.sync.dma_start(
                                t[:, 2 * i2 : 2 * i2 + 2, :],
                                dram[:, 2 * i2 : 2 * i2 + 2, s_lo : s_lo + QB],
                            )
                    else:
                        nc.sync.dma_start(t, dram[:, :, s_lo : s_lo + QB])
                    tiles[tg] = t
                return tiles["xk"], tiles["xq"], tiles["xv"]

            def proj_block(qb, x_tiles):
                s_lo = qb * QB
                xk_t, xq_t, xv_t = x_tiles

                def kq_proj_group(x_t, w_sb, b_sb, dst, c2):
                    ps = pp.tile([128, QB], F32, tag="pp", name="ps")
                    for dc in range(dch):
                        nc.tensor.matmul(
                            ps,
                            lhsT=w_sb[:, dc, c2 * 128 : (c2 + 1) * 128],
                            rhs=x_t[:, dc, 0:QB],
                            start=(dc == 0),
                            stop=(dc == dch - 1),
                        )
                    nc.vector.tensor_scalar_add(
                        dst[:, c2, s_lo : s_lo + QB], ps, b_sb[:, c2 : c2 + 1]
                    )

                def v_proj_group(sc):
                    sck = qb * nqc + sc
                    ps = pp.tile([128, hd], F32, tag="pp", name="ps")
                    for dc in range(dch):
                        nc.tensor.matmul(
                            ps,
                            lhsT=xv_t[:, dc, sc * 128 : (sc + 1) * 128],
                            rhs=wv_sb[:, dc, :],
                            start=(dc == 0),
                            stop=(dc == dch - 1),
                        )
                    nc.vector.tensor_copy(
                        v_sb[:, sck, :].rearrange("p (h j) -> p h j", j=65)[:, :, 0:64],
                        ps[:].rearrange("p (h j) -> p h j", j=64),
                    )

                return kq_proj_group, v_proj_group, xk_t, xq_t

            def proj_q(groups):
                kq_proj_group, _, _, xq_t = groups
                for c2 in range(npair):
                    kq_proj_group(xq_t, wq_sb, bq_sb, qT, c2)

            def proj_k(groups):
                kq_proj_group, _, xk_t, _ = groups
                for c2 in range(npair):
                    kq_proj_group(xk_t, wk_sb, bk_sb, kT, c2)

            def proj_v(groups):
                _, v_proj_group, _, _ = groups
                for sc in range(nqc):
                    v_proj_group(sc)

            def attn_block(qb, a_sb, mid_emit=None, tail_emit=None,
                           load_emit=None, borrow_pp=False):
                s_lo = qb * QB
                n_chunks = (qb + 1) * nqc if mask_mode == "causal" else nkc
                diag_lo = qb * nqc
                assert n_chunks % 2 == 0
                if mask_mode == "general":
                    m_tiles = []
                    for kc_i in range(n_chunks):
                        mt = mpool.tile([128, QB], BF16, tag="m")
                        nc.sync.dma_start(
                            mt, m01[kc_i * KC : (kc_i + 1) * KC, s_lo : s_lo + QB]
                        )
                        m_tiles.append(mt)

                def pv(a_ps, qci, kcp, e_cp):
                    """accumulate A[qci] over the two chunks of pair kcp.
                    start/stop stay False: a PSUM start flag zeroes the
                    whole 2KB bank, so the four per-head chains in one bank
                    are bracketed by full-width zero/stop matmuls instead
                    (see a_zero/a_stop) whose region overlap also pins the
                    scheduler ordering."""
                    qcg = diag_lo + qci
                    last = qcg if mask_mode == "causal" else n_chunks - 1
                    for ck in range(2):
                        kc_i = kcp + ck
                        if mask_mode == "causal" and kc_i > qcg:
                            continue
                        for h in range(heads):
                            # PSUM start zeroes the whole 2KB bank lazily
                            # (first write per byte range overwrites), so
                            # only the bank's first instruction carries
                            # start and only its last carries stop; the
                            # scheduler's deterministic priority order
                            # keeps h0/kc0 first (validated by CoreSim's
                            # zero-region checks on the frozen schedule).
                            nc.tensor.matmul(
                                a_ps[:, h * 65 : (h + 1) * 65],
                                lhsT=e_cp[h][:, ck * QB + qci * KC : ck * QB + (qci + 1) * KC],
                                rhs=v_sb[:, kc_i, h * 65 : (h + 1) * 65],
                                start=(kc_i == 0 and h == 0),
                                stop=(kc_i == last and h == heads - 1),
                            )


                e_store = []
                if load_emit is not None:
                    # next block's x fetch issues now so the transfers hide
                    # under this sweep instead of gating the next q-proj
                    load_emit()
                mid_done = mid_emit is None
                # borrow_pp=True: the second half's accumulators come from
                # the pp pool (requested after this block's k/v projection
                # groups, so the pp FIFO stays deadlock-free).  Legal only
                # when no LATER projections will need pp.
                for half, qcis in enumerate(((0, 1), (2, 3))):
                    a_ps = {}
                    for qci in qcis:
                        pool = pp if (half == 1 and borrow_pp) else ap2
                        tg = "pp" if (half == 1 and borrow_pp) else "A"
                        a_ps[qci] = pool.tile(
                            [128, heads * 65], F32, tag=tg, name=f"aps{qci}"
                        )
                    for cp_i, kcp in enumerate(range(0, n_chunks, 2)):
                        if half == 0:
                            # emit this block's k/v projections just before
                            # the first diagonal chunk-pair needs them: the
                            # pp-pool FIFO and scheduler priorities then
                            # match the time order of needs
                            if not mid_done and kcp + 1 >= diag_lo:
                                mid_emit()
                                mid_done = True
                            # --- scores + exp for both head pairs; k-chunk
                            # PAIRS share one [128, 2*QB] psum tile (2 banks)
                            # so exp runs as a single wide ACT op ---
                            if mask_mode == "causal":
                                skips = [
                                    max(0, ((kcp + ck) - diag_lo) * KC)
                                    if (kcp + ck) >= diag_lo
                                    else 0
                                    for ck in range(2)
                                ]
                            else:
                                skips = [0, 0]
                            e_cp = [None] * heads
                            for pr in range(npair):
                                s_pss = [
                                    sp.tile([128, 2 * QB], F32, tag="sp", name=f"sps{sub}")
                                    for sub in range(2)
                                ]
                                # emit the four score matmuls alternating
                                # head-subs: adjacent MMs then occupy disjoint
                                # PE row-groups (partitions 0-63 vs 64-127)
                                for ck in range(2):
                                    sk = skips[ck]
                                    for sub in range(2):
                                        rows = slice(sub * 64, sub * 64 + 64)
                                        nc.tensor.matmul(
                                            s_pss[sub][:, ck * QB + sk : (ck + 1) * QB],
                                            lhsT=kT[rows, pr, (kcp + ck) * KC : (kcp + ck + 1) * KC],
                                            rhs=qT[rows, pr, s_lo + sk : s_lo + QB],
                                            start=True,
                                            stop=True,
                                        )
                                for sub in range(2):
                                    s_ps = s_pss[sub]
                                    e = epool.tile([128, 2 * QB], BF16, tag="e")
                                    if skips[0] == 0 and skips[1] == 0:
                                        nc.scalar.activation(
                                            out=e, in_=s_ps,
                                            func=mybir.ActivationFunctionType.Exp,
                                        )
                                    else:
                                        for ck in range(2):
                                            sk = skips[ck]
                                            nc.scalar.activation(
                                                out=e[:, ck * QB + sk : (ck + 1) * QB],
                                                in_=s_ps[:, ck * QB + sk : (ck + 1) * QB],
                                                func=mybir.ActivationFunctionType.Exp,
                                            )
                                    for ck in range(2):
                                        kc_i = kcp + ck
                                        sk = skips[ck]
                                        if mask_mode == "causal" and kc_i >= diag_lo:
                                            # keep condition is (f' - p) >= 0
                                            nc.gpsimd.affine_select(
                                                out=e[:, ck * QB + sk : (ck + 1) * QB],
                                                in_=e[:, ck * QB + sk : (ck + 1) * QB],
                                                compare_op=mybir.AluOpType.is_ge,
                                                fill=0.0,
                                                base=0,
                                                pattern=[[1, QB - sk]],
                                                channel_multiplier=-1,
                                            )
                                        if mask_mode == "general":
                                            nc.vector.tensor_mul(
                                                e[:, ck * QB : (ck + 1) * QB],
                                                e[:, ck * QB : (ck + 1) * QB],
                                                m_tiles[kc_i],
                                            )
                                    e_cp[pr * 2 + sub] = e
                            e_store.append(e_cp)
                        for qci in qcis:
                            pv(a_ps[qci], qci, kcp, e_store[cp_i])
                    if half == 0 and not mid_done:
                        mid_emit()
                        mid_done = True
                    # --- normalize + transpose each finished q chunk ---
                    for qci in qcis:
                        aps = a_ps[qci]
                        aps_r = aps[:].rearrange("p (h j) -> p h j", j=65)
                        rec = rpool.tile([128, heads, 1], F32, tag="r")
                        nc.vector.reciprocal(rec, aps_r[:, :, 64:65])
                        a_n = anp.tile([128, hd], BF16, tag="an")
                        for h in range(heads):
                            nc.vector.tensor_scalar_mul(
                                a_n[:, h * 64 : (h + 1) * 64],
                                aps_r[:, h, 0:64],
                                rec[:, h, :],
                            )
                        for pr in range(npair):
                            nc.sync.dma_start(
                                a_sb[:, pr, qci * KC : (qci + 1) * KC],
                                a_n[:, pr * 128 : (pr + 1) * 128],
                                transpose=True,
                            )
                    if half == 0 and tail_emit is not None:
                        # next block's q projection (and x fetch) outranks
                        # this block's second-half PV replay: it is the
                        # critical path of the next sweep
                        tail_emit()

            def out_proj(qb, a_sb, tail=False):
                s_lo = qb * QB
                for qcr in range(nqc):
                    o_sb = opool.tile([128, d], BF16, tag="o")
                    for nb in range(d // QB):
                        # ap2 (not sp): during sweeps the scores own sp, so
                        # out-proj would starve until the sweep boundary;
                        # ap2 slots free right after the norms instead.
                        ops = ap2.tile([128, QB], F32, tag="A", name="ops")
                        for c2 in range(npair):
                            nc.tensor.matmul(
                                ops,
                                lhsT=a_sb[:, c2, qcr * KC : (qcr + 1) * KC],
                                rhs=wo_sb[:, c2, nb * QB : (nb + 1) * QB],
                                start=(c2 == 0),
                                stop=(c2 == npair - 1),
                            )
                        # DVE (the Pool engine has no PSUM port); in the
                        # tail the Act engine is idle, split with it there
                        if tail and nb % 2 == 1:
                            nc.scalar.copy(
                                o_sb[:, nb * QB : (nb + 1) * QB], ops
                            )
                        else:
                            nc.vector.tensor_copy(
                                o_sb[:, nb * QB : (nb + 1) * QB], ops
                            )
                    nc.sync.dma_start(
                        out[(s_lo + qcr * KC) : (s_lo + (qcr + 1) * KC), :], o_sb
                    )

            if mask_mode == "causal":
                # attention(qb) only reads k/v ranges projected so far.
                # Emission order sets scheduler priority AND the FIFO grant
                # order of the psum pools, so it must match the time order
                # of needs: q(qb) -> sweep(qb) non-diag -> k/v(qb) ->
                # sweep(qb) diag -> q(qb+1) -> PV replay -> out-proj(qb).
                groups_of = {}

                def make_load(nqb):
                    def load():
                        groups_of[nqb] = proj_block(nqb, load_x(nqb, fine=False))
                    return load

                def make_tail(nqb):
                    def tail():
                        proj_q(groups_of[nqb])
                    return tail

                def make_mid(qb):
                    def mid():
                        proj_k(groups_of[qb])
                        proj_v(groups_of[qb])
                    return mid

                groups_of[0] = proj_block(0, load_x(0, fine=True))
                proj_k(groups_of[0])
                proj_q(groups_of[0])
                proj_v(groups_of[0])
                for qb in range(nq):
                    a_sb = atp.tile([128, npair, QB], BF16, tag="aT")
                    attn_block(
                        qb,
                        a_sb,
                        mid_emit=(make_mid(qb) if qb > 0 else None),
                        tail_emit=(make_tail(qb + 1) if qb + 1 < nq else None),
                        load_emit=(make_load(qb + 1) if qb + 1 < nq else None),
                        borrow_pp=(qb == nq - 1),
                    )
                    out_proj(qb, a_sb, tail=(qb == nq - 1))
            else:
                # unmasked attention reads ALL k/v: project everything first
                # (pp is idle during attention, so every block can borrow)
                for qb in range(nq):
                    groups = proj_block(qb, load_x(qb, fine=(qb == 0)))
                    proj_k(groups)
                    proj_q(groups)
                    proj_v(groups)
                for qb in range(nq):
                    a_sb = atp.tile([128, npair, QB], BF16, tag="aT")
                    attn_block(qb, a_sb, borrow_pp=True)
                    out_proj(qb, a_sb)

    return nc


# ---------------------------------------------------------------------------
# host side
# ---------------------------------------------------------------------------

_PROG_CACHE = {}
LAST_RESULT = None


def _get_program(mask_mode):
    if mask_mode not in _PROG_CACHE:
        _PROG_CACHE[mask_mode] = build_program(mask_mode)
    return _PROG_CACHE[mask_mode]


def _bf16(a):
    return np.ascontiguousarray(a).astype(BF16_NP)


def _pack_w(wT):
    """[D, m] -> [128, (D//128)*m] with row p holding chunks contiguously."""
    dch_, m = wT.shape[0] // 128, wT.shape[1]
    return np.ascontiguousarray(
        wT.reshape(dch_, 128, m).transpose(1, 0, 2).reshape(128, dch_ * m)
    )


def _pack_b(b):
    """[HD] -> [128, npair]: column c2 holds partitions of head-pair c2."""
    npair = HD // 128
    return np.ascontiguousarray(b.reshape(npair, 128).T).astype(np.float32)


def kernel(query, key_in, value, mask, w_q, b_q, w_k, b_k, w_v, b_v, w_out, b_out):
    from concourse.bass_utils import run_bass_kernel_spmd

    query = np.asarray(query, dtype=np.float32)
    key_in = np.asarray(key_in, dtype=np.float32)
    value = np.asarray(value, dtype=np.float32)
    mask = np.asarray(mask)
    w_q = np.asarray(w_q, dtype=np.float32)
    b_q = np.asarray(b_q, dtype=np.float32)
    w_k = np.asarray(w_k, dtype=np.float32)
    b_k = np.asarray(b_k, dtype=np.float32)
    w_v = np.asarray(w_v, dtype=np.float32)
    b_v = np.asarray(b_v, dtype=np.float32)
    w_out = np.asarray(w_out, dtype=np.float32)
    b_out = np.asarray(b_out, dtype=np.float32)

    scale = 1.0 / np.sqrt(np.float32(D_K))

    if (mask == 1).all():
        mode = "ones"
    elif all(
        np.array_equal(mask[b, 0], np.tril(np.ones((S, S), mask.dtype)))
        for b in range(mask.shape[0])
    ):
        mode = "causal"
    else:
        mode = "general"
    nc = _get_program(mode)

    wqT = _bf16(w_q.T * scale)   # [D, D] scaled
    wkT = _bf16(w_k.T)
    wvT = _bf16(w_v.T)
    woT = _bf16(w_out.T)
    bq_s = (b_q * scale).astype(np.float32)

    # per-batch transposed activations, shared by the 4 cores of a batch
    xqT = [_bf16(query[b].T) for b in range(B)]
    xkT = [_bf16(key_in[b].T) for b in range(B)]
    xvT = [_bf16(value[b].T) for b in range(B)]
    m01T = [_bf16(mask[b, 0].T) for b in range(B)] if mode == "general" else None

    in_maps = []
    for c in range(N_CORES):
        b = c // CORES_PER_BATCH
        hg = c % CORES_PER_BATCH
        hsl = slice(hg * HD, (hg + 1) * HD)
        im = {
            "xq": xqT[b],
            "xk": xkT[b],
            "xv": xvT[b],
            "wq": _pack_w(wqT[:, hsl]),
            "wk": _pack_w(wkT[:, hsl]),
            "wv": _pack_w(wvT[:, hsl]),
            "wo": _pack_w(woT[hsl, :]),
            "bq": _pack_b(bq_s[hsl]),
            "bk": _pack_b(b_k[hsl]),
        }
        if mode == "general":
            im["m01"] = m01T[b]
        in_maps.append(im)

    global LAST_RESULT
    try:
        res = run_bass_kernel_spmd(nc, in_maps, list(range(N_CORES)))
    except Exception:
        # transient NRT_EXEC_UNIT_UNRECOVERABLE wedges have been observed on
        # this fabric; a single retry has always cleared them
        import time as _time

        _time.sleep(3.0)
        res = run_bass_kernel_spmd(nc, in_maps, list(range(N_CORES)))
    LAST_RESULT = res

    b_eff = b_out + w_out @ b_v
    out = np.zeros((B, S, D_MODEL), dtype=np.float32)
    for c in range(N_CORES):
        out[c // CORES_PER_BATCH] += res.results[c]["out"].astype(np.float32)
    out += b_eff[None, None, :]
    return out


# revision 45
# speedup vs baseline: 1.0231x; 1.0014x over previous
"""Multi-head attention block on 8 Trainium2 NeuronCores.

Sharding: batch (B=2) x head-groups (4 heads each) -> 8 cores.
Each core computes q/k/v projections for its 4 heads of its batch,
causal attention, and a partial output projection; the host sums the 4
partials per batch and adds the bias.

Layout: projections produce qT/kT ([head_dim, seq]) and v in natural
per-chunk layout ([seq-chunk, head_dim] with a ones column per head for
the softmax denominator).  Scores are computed transposed (S^T [k, q])
so exp/masking work on [k, q] tiles, then the PV matmul flips back to
natural A [q, head_dim(+denom)] with the q tokens on the partition axis
(m=128, n=65/head: half the moving-dim cycles of the [65, q] form).
The denominator lands per-partition, so normalization is a cheap
per-partition tensor_scalar multiply; the normalized A is transposed
back to [head_dim, q] for the output projection with the DMA XBAR
transpose (off the PE entirely).
Biases: b_q/b_k applied on-chip (per-partition); b_v and b_out folded
into a host-side constant (b_out + w_out @ b_v) added after gathering.
"""

import os
import re
import sys

sys.path.insert(0, "/opt/trn_rl_repo")

import numpy as np
import ml_dtypes

import concourse.bass as bass
import concourse.mybir as mybir
import concourse.tile as tile

BF16 = mybir.dt.bfloat16
F32 = mybir.dt.float32
BF16_NP = ml_dtypes.bfloat16

N_CORES = 8
B = 2
S = 2048
D_MODEL = 1024
H_TOTAL = 16
D_K = 64
H_PER_CORE = 4                      # heads per core
HD = H_PER_CORE * D_K               # 256 head-dims per core
CORES_PER_BATCH = N_CORES // B

QB = 512                            # q-block (matmul moving free dim)
KC = 128                            # k-chunk (contraction tile)

N_PROCS = 27


def _split_waits_json(bir_json: bytes, limit: int = 1) -> bytes:
    """walrus in this env rejects >limit sync-waits on an instruction.
    Hoist excess waits onto fresh NoOps inserted just before, on the same
    engine queue (queue execution is serial, so ordering is identical)."""
    import orjson

    m = orjson.loads(bir_json)
    ctr = 0
    for fn in m.get("functions", []):
        for bb in fn.get("blocks", []):
            insts = bb.get("instructions") or []
            if not any(
                len((i.get("sync_info") or {}).get("on_wait") or []) > limit
                for i in insts
            ):
                continue
            out = []
            for inst in insts:
                si = inst.get("sync_info")
                waits = (si or {}).get("on_wait") or []
                if len(waits) > limit:
                    for w in waits[:-limit]:
                        ctr += 1
                        out.append(
                            {
                                "debug": inst.get("debug", 0),
                                "engine": inst["engine"],
                                "ins": [],
                                "outs": [],
                                "name": f"WSPL-{ctr}",
                                "opcode": "NoOp",
                                "sync_info": {"on_update": [], "on_wait": [w]},
                            }
                        )
                    si["on_wait"] = waits[-limit:]
                out.append(inst)
            bb["instructions"] = out
    return orjson.dumps(m)


LAST_PREDICTED_NS = None


def _install_schedule_capture():
    """Record the Tile scheduler's cost-model makespan for each build."""
    if getattr(tile.TileContext, "_capture_installed", False):
        return
    orig = tile.TileContext.schedule_block

    def wrapped(self, *a, **kw):
        r = orig(self, *a, **kw)
        try:
            global LAST_PREDICTED_NS
            LAST_PREDICTED_NS = r[1].time
        except Exception:
            pass
        return r

    tile.TileContext.schedule_block = wrapped
    tile.TileContext._capture_installed = True


def _install_compile_patch():
    import concourse.bass_utils as bu
    import concourse.bass2jax as b2j

    if getattr(bu, "_wait_split_installed", False):
        return
    orig = bu.compile_bir_kernel

    def wrapped(bir_json, tmpdir, neff_name="file.neff"):
        return orig(_split_waits_json(bytes(bir_json)), tmpdir, neff_name)

    bu.compile_bir_kernel = wrapped
    b2j.compile_bir_kernel = wrapped
    bu._wait_split_installed = True


def build_program(mask_mode="causal", s=S, d=D_MODEL, heads=H_PER_CORE,
                  epool_bufs=34, opool_bufs=3):
    """One SPMD program; per-core behavior differs only via inputs.

    mask_mode: "causal" (skip above-diagonal chunks, affine-select the
    diagonal ones), "ones" (no masking), "general" (multiplicative 0/1
    mask loaded from DRAM, pre-transposed host-side).
    """
    _install_compile_patch()
    _install_schedule_capture()
    hd = heads * D_K
    nq = s // QB          # q blocks
    nkc = s // KC         # k chunks
    nqc = QB // KC        # q chunks per q block
    dch = d // 128        # contraction chunks for projections
    npair = heads // 2    # head pairs (even head on partitions 0-63)
    assert hd % 128 == 0 and hd // 128 == npair

    nc = bass.Bass()
    xq = nc.dram_tensor("xq", [d, s], BF16, kind="ExternalInput")
    xk = nc.dram_tensor("xk", [d, s], BF16, kind="ExternalInput")
    xv = nc.dram_tensor("xv", [d, s], BF16, kind="ExternalInput")
    # weights arrive pre-packed host-side so each partition's line is one
    # contiguous 4KB run: wq[p, c*hd+m] = w_q.T[c*128+p, m] etc.
    wq = nc.dram_tensor("wq", [128, dch * hd], BF16, kind="ExternalInput")
    wk = nc.dram_tensor("wk", [128, dch * hd], BF16, kind="ExternalInput")
    wv = nc.dram_tensor("wv", [128, dch * hd], BF16, kind="ExternalInput")
    wo = nc.dram_tensor("wo", [128, npair * d], BF16, kind="ExternalInput")
    bq = nc.dram_tensor("bq", [128, npair], F32, kind="ExternalInput")
    bk = nc.dram_tensor("bk", [128, npair], F32, kind="ExternalInput")
    if mask_mode == "general":
        m01 = nc.dram_tensor("m01", [s, s], BF16, kind="ExternalInput")
    out = nc.dram_tensor("out", [s, d], BF16, kind="ExternalOutput")

    xq_r = xq[:, :].rearrange("(c p) s -> p c s", p=128)
    xk_r = xk[:, :].rearrange("(c p) s -> p c s", p=128)
    xv_r = xv[:, :].rearrange("(c p) s -> p c s", p=128)
    wq_r = wq[:, :].rearrange("p (c m) -> p c m", m=hd)
    wk_r = wk[:, :].rearrange("p (c m) -> p c m", m=hd)
    wv_r = wv[:, :].rearrange("p (c m) -> p c m", m=hd)
    wo_r = wo[:, :].rearrange("p (c e) -> p c e", e=d)

    with tile.TileContext(nc) as tc:
        with (
            tc.tile_pool(name="consts", bufs=1) as consts,
            tc.tile_pool(name="qkres", bufs=1) as qkres,
            tc.tile_pool(name="xkp", bufs=2) as xkp,
            tc.tile_pool(name="xqp", bufs=2) as xqp,
            tc.tile_pool(name="xvp", bufs=2) as xvp,
            tc.tile_pool(name="epool", bufs=epool_bufs) as epool,
            tc.tile_pool(name="anp", bufs=4) as anp,
            tc.tile_pool(name="atp", bufs=2) as atp,
            tc.tile_pool(name="opool", bufs=opool_bufs) as opool,
            tc.tile_pool(name="rpool", bufs=4) as rpool,
            tc.tile_pool(name="mpool", bufs=16) as mpool,
            tc.tile_pool(name="pp", bufs=2, space="PSUM") as pp,
            tc.tile_pool(name="sp", bufs=2, space="PSUM") as sp,
            tc.tile_pool(name="ap2", bufs=2, space="PSUM") as ap2,
        ):
            # --- weights + biases on the Activation DMA queue; x streams
            # on SP so both queues issue in parallel at startup.  wk is
            # split so the very first projection matmul only waits for a
            # small transfer.
            wk_sb = consts.tile([128, dch, hd], BF16, tag="wk")
            nc.sync.dma_start(wk_sb[:, 0:2, :], wk_r[:, 0:2, :])
            # PE warm-up: dummy matmuls ramp the tensor engine out of its
            # low p-state while the first input DMAs are still in flight.
            warm = consts.tile([1, QB], BF16, tag="warm")
            nc.vector.memset(warm, 0.0)
            wps = sp.tile([128, 2 * QB], F32, tag="sp", name="warmps")
            for _ in range(6):
                nc.tensor.matmul(
                    wps[:, 0:QB], lhsT=warm[:, 0:128], rhs=warm,
                    start=True, stop=True,
                )
            nc.sync.dma_start(wk_sb[:, 2:dch, :], wk_r[:, 2:dch, :])
            bk_sb = consts.tile([128, npair], F32, tag="bk")
            nc.scalar.dma_start(bk_sb, bk[:, :])
            bq_sb = consts.tile([128, npair], F32, tag="bq")
            nc.scalar.dma_start(bq_sb, bq[:, :])
            wq_sb = consts.tile([128, dch, hd], BF16, tag="wq")
            nc.scalar.dma_start(wq_sb, wq_r)
            wv_sb = consts.tile([128, dch, hd], BF16, tag="wv")
            nc.scalar.dma_start(wv_sb, wv_r)
            wo_sb = consts.tile([128, npair, d], BF16, tag="wo")
            nc.scalar.dma_start(wo_sb, wo_r)

            # persistent per-core tensors
            qT = qkres.tile([128, npair, s], BF16, tag="qT")
            kT = qkres.tile([128, npair, s], BF16, tag="kT")
            v_sb = qkres.tile([128, nkc, heads * 65], BF16, tag="v")
            # fill with 1.0 once; v-proj copies overwrite cols 0:64 of each
            # 65-block, leaving column 64 = 1.0 (softmax denominator trick)
            nc.vector.memset(v_sb, 1.0)

            def load_x(qb, fine):
                """Fetch the qb-th quarter of xk/xq/xv into rotating tiles.
                fine=True splits into 2-dc pieces so the first projections
                can start before the whole quarter has landed."""
                s_lo = qb * QB
                tiles = {}
                order = (
                    ((xkp, xk_r, "xk"), (xqp, xq_r, "xq"), (xvp, xv_r, "xv"))
                    if fine
                    else ((xqp, xq_r, "xq"), (xkp, xk_r, "xk"), (xvp, xv_r, "xv"))
                )
                for pool, dram, tg in order:
                    t = pool.tile([128, dch, QB], BF16, tag=tg)
                    if fine:
                        for i2 in range(dch // 2):
                            nc.sync.dma_start(
                                t[:, 2 * i2 : 2 * i2 + 2, :],
                                dram[:, 2 * i2 : 2 * i2 + 2, s_lo : s_lo + QB],
                            )
                    else:
                        nc.sync.dma_start(t, dram[:, :, s_lo : s_lo + QB])
                    tiles[tg] = t
                return tiles["xk"], tiles["xq"], tiles["xv"]

            def proj_block(qb, x_tiles):
                s_lo = qb * QB
                xk_t, xq_t, xv_t = x_tiles

                def kq_proj_group(x_t, w_sb, b_sb, dst, c2):
                    ps = pp.tile([128, QB], F32, tag="pp", name="ps")
                    for dc in range(dch):
                        nc.tensor.matmul(
                            ps,
                            lhsT=w_sb[:, dc, c2 * 128 : (c2 + 1) * 128],
                            rhs=x_t[:, dc, 0:QB],
                            start=(dc == 0),
                            stop=(dc == dch - 1),
                        )
                    nc.vector.tensor_scalar_add(
                        dst[:, c2, s_lo : s_lo + QB], ps, b_sb[:, c2 : c2 + 1]
                    )

                def v_proj_group(sc):
                    sck = qb * nqc + sc
                    ps = pp.tile([128, hd], F32, tag="pp", name="ps")
                    for dc in range(dch):
                        nc.tensor.matmul(
                            ps,
                            lhsT=xv_t[:, dc, sc * 128 : (sc + 1) * 128],
                            rhs=wv_sb[:, dc, :],
                            start=(dc == 0),
                            stop=(dc == dch - 1),
                        )
                    nc.vector.tensor_copy(
                        v_sb[:, sck, :].rearrange("p (h j) -> p h j", j=65)[:, :, 0:64],
                        ps[:].rearrange("p (h j) -> p h j", j=64),
                    )

                return kq_proj_group, v_proj_group, xk_t, xq_t

            def proj_q(groups):
                kq_proj_group, _, _, xq_t = groups
                for c2 in range(npair):
                    kq_proj_group(xq_t, wq_sb, bq_sb, qT, c2)

            def proj_k(groups):
                kq_proj_group, _, xk_t, _ = groups
                for c2 in range(npair):
                    kq_proj_group(xk_t, wk_sb, bk_sb, kT, c2)

            def proj_v(groups):
                _, v_proj_group, _, _ = groups
                for sc in range(nqc):
                    v_proj_group(sc)

            def attn_block(qb, a_sb, mid_emit=None, tail_emit=None,
                           load_emit=None, borrow_pp=False):
                s_lo = qb * QB
                n_chunks = (qb + 1) * nqc if mask_mode == "causal" else nkc
                diag_lo = qb * nqc
                assert n_chunks % 2 == 0
                if mask_mode == "general":
                    m_tiles = []
                    for kc_i in range(n_chunks):
                        mt = mpool.tile([128, QB], BF16, tag="m")
                        nc.sync.dma_start(
                            mt, m01[kc_i * KC : (kc_i + 1) * KC, s_lo : s_lo + QB]
                        )
                        m_tiles.append(mt)

                def pv(a_ps, qci, kcp, e_cp):
                    """accumulate A[qci] over the two chunks of pair kcp.
                    start/stop stay False: a PSUM start flag zeroes the
                    whole 2KB bank, so the four per-head chains in one bank
                    are bracketed by full-width zero/stop matmuls instead
                    (see a_zero/a_stop) whose region overlap also pins the
                    scheduler ordering."""
                    qcg = diag_lo + qci
                    last = qcg if mask_mode == "causal" else n_chunks - 1
                    for ck in range(2):
                        kc_i = kcp + ck
                        if mask_mode == "causal" and kc_i > qcg:
                            continue
                        for h in range(heads):
                            # PSUM start zeroes the whole 2KB bank lazily
                            # (first write per byte range overwrites), so
                            # only the bank's first instruction carries
                            # start and only its last carries stop; the
                            # scheduler's deterministic priority order
                            # keeps h0/kc0 first (validated by CoreSim's
                            # zero-region checks on the frozen schedule).
                            nc.tensor.matmul(
                                a_ps[:, h * 65 : (h + 1) * 65],
                                lhsT=e_cp[h][:, ck * QB + qci * KC : ck * QB + (qci + 1) * KC],
                                rhs=v_sb[:, kc_i, h * 65 : (h + 1) * 65],
                                start=(kc_i == 0 and h == 0),
                                stop=(kc_i == last and h == heads - 1),
                            )


                e_store = []
                if load_emit is not None:
                    # next block's x fetch issues now so the transfers hide
                    # under this sweep instead of gating the next q-proj
                    load_emit()
                mid_done = mid_emit is None
                # borrow_pp=True: the second half's accumulators come from
                # the pp pool (requested after this block's k/v projection
                # groups, so the pp FIFO stays deadlock-free).  Legal only
                # when no LATER projections will need pp.
                for half, qcis in enumerate(((0, 1), (2, 3))):
                    a_ps = {}
                    for qci in qcis:
                        pool = pp if (half == 1 and borrow_pp) else ap2
                        tg = "pp" if (half == 1 and borrow_pp) else "A"
                        a_ps[qci] = pool.tile(
                            [128, heads * 65], F32, tag=tg, name=f"aps{qci}"
                        )
                    for cp_i, kcp in enumerate(range(0, n_chunks, 2)):
                        if half == 0:
                            # emit this block's k/v projections just before
                            # the first diagonal chunk-pair needs them: the
                            # pp-pool FIFO and scheduler priorities then
                            # match the time order of needs
                            if not mid_done and kcp + 1 >= diag_lo:
                                mid_emit()
                                mid_done = True
                            # --- scores + exp for both head pairs; k-chunk
                            # PAIRS share one [128, 2*QB] psum tile (2 banks)
                            # so exp runs as a single wide ACT op ---
                            if mask_mode == "causal":
                                skips = [
                                    max(0, ((kcp + ck) - diag_lo) * KC)
                                    if (kcp + ck) >= diag_lo
                                    else 0
                                    for ck in range(2)
                                ]
                            else:
                                skips = [0, 0]
                            e_cp = [None] * heads
                            for pr in range(npair):
                                s_pss = [
                                    sp.tile([128, 2 * QB], F32, tag="sp", name=f"sps{sub}")
                                    for sub in range(2)
                                ]
                                # emit the four score matmuls alternating
                                # head-subs: adjacent MMs then occupy disjoint
                                # PE row-groups (partitions 0-63 vs 64-127)
                                for ck in range(2):
                                    sk = skips[ck]
                                    for sub in range(2):
                                        rows = slice(sub * 64, sub * 64 + 64)
                                        nc.tensor.matmul(
                                            s_pss[sub][:, ck * QB + sk : (ck + 1) * QB],
                                            lhsT=kT[rows, pr, (kcp + ck) * KC : (kcp + ck + 1) * KC],
                                            rhs=qT[rows, pr, s_lo + sk : s_lo + QB],
                                            start=True,
                                            stop=True,
                                        )
                                for sub in range(2):
                                    s_ps = s_pss[sub]
                                    e = epool.tile([128, 2 * QB], BF16, tag="e")
                                    if skips[0] == 0 and skips[1] == 0:
                                        nc.scalar.activation(
                                            out=e, in_=s_ps,
                                            func=mybir.ActivationFunctionType.Exp,
                                        )
                                    else:
                                        for ck in range(2):
                                            sk = skips[ck]
                                            nc.scalar.activation(
                                                out=e[:, ck * QB + sk : (ck + 1) * QB],
                                                in_=s_ps[:, ck * QB + sk : (ck + 1) * QB],
                                                func=mybir.ActivationFunctionType.Exp,
                                            )
                                    for ck in range(2):
                                        kc_i = kcp + ck
                                        sk = skips[ck]
                                        if mask_mode == "causal" and kc_i >= diag_lo:
                                            # keep condition is (f' - p) >= 0
                                            nc.gpsimd.affine_select(
                                                out=e[:, ck * QB + sk : (ck + 1) * QB],
                                                in_=e[:, ck * QB + sk : (ck + 1) * QB],
                                                compare_op=mybir.AluOpType.is_ge,
                                                fill=0.0,
                                                base=0,
                                                pattern=[[1, QB - sk]],
                                                channel_multiplier=-1,
                                            )
                                        if mask_mode == "general":
                                            nc.vector.tensor_mul(
                                                e[:, ck * QB : (ck + 1) * QB],
                                                e[:, ck * QB : (ck + 1) * QB],
                                                m_tiles[kc_i],
                                            )
                                    e_cp[pr * 2 + sub] = e
                            e_store.append(e_cp)
                        for qci in qcis:
                            pv(a_ps[qci], qci, kcp, e_store[cp_i])
                    if half == 0 and not mid_done:
                        mid_emit()
                        mid_done = True
                    # --- normalize + transpose each finished q chunk ---
                    for qci in qcis:
                        aps = a_ps[qci]
                        aps_r = aps[:].rearrange("p (h j) -> p h j", j=65)
                        rec = rpool.tile([128, heads, 1], F32, tag="r")
                        nc.vector.reciprocal(rec, aps_r[:, :, 64:65])
                        a_n = anp.tile([128, hd], BF16, tag="an")
                        for h in range(heads):
                            nc.vector.tensor_scalar_mul(
                                a_n[:, h * 64 : (h + 1) * 64],
                                aps_r[:, h, 0:64],
                                rec[:, h, :],
                            )
                        for pr in range(npair):
                            # final block: issue from the Act queue so the
                            # transposes don't serialize behind out-DMAs on
                            # the SP queue at the kernel tail
                            eng = nc.scalar if (borrow_pp and half == 1) else nc.sync
                            eng.dma_start(
                                a_sb[:, pr, qci * KC : (qci + 1) * KC],
                                a_n[:, pr * 128 : (pr + 1) * 128],
                                transpose=True,
                            )
                    if half == 0 and tail_emit is not None:
                        # next block's q projection (and x fetch) outranks
                        # this block's second-half PV replay: it is the
                        # critical path of the next sweep
                        tail_emit()

            def out_proj(qb, a_sb, tail=False):
                s_lo = qb * QB
                for qcr in range(nqc):
                    o_sb = opool.tile([128, d], BF16, tag="o")
                    for nb in range(d // QB):
                        # ap2 (not sp): during sweeps the scores own sp, so
                        # out-proj would starve until the sweep boundary;
                        # ap2 slots free right after the norms instead.
                        ops = ap2.tile([128, QB], F32, tag="A", name="ops")
                        for c2 in range(npair):
                            nc.tensor.matmul(
                                ops,
                                lhsT=a_sb[:, c2, qcr * KC : (qcr + 1) * KC],
                                rhs=wo_sb[:, c2, nb * QB : (nb + 1) * QB],
                                start=(c2 == 0),
                                stop=(c2 == npair - 1),
                            )
                        # DVE (the Pool engine has no PSUM port); in the
                        # tail the Act engine is idle, split with it there
                        if tail and nb % 2 == 1:
                            nc.scalar.copy(
                                o_sb[:, nb * QB : (nb + 1) * QB], ops
                            )
                        else:
                            nc.vector.tensor_copy(
                                o_sb[:, nb * QB : (nb + 1) * QB], ops
                            )
                    nc.sync.dma_start(
                        out[(s_lo + qcr * KC) : (s_lo + (qcr + 1) * KC), :], o_sb
                    )

            if mask_mode == "causal":
                # attention(qb) only reads k/v ranges projected so far.
                # Emission order sets scheduler priority AND the FIFO grant
                # order of the psum pools, so it must match the time order
                # of needs: q(qb) -> sweep(qb) non-diag -> k/v(qb) ->
                # sweep(qb) diag -> q(qb+1) -> PV replay -> out-proj(qb).
                groups_of = {}

                def make_load(nqb):
                    def load():
                        groups_of[nqb] = proj_block(nqb, load_x(nqb, fine=False))
                    return load

                def make_tail(nqb):
                    def tail():
                        proj_q(groups_of[nqb])
                    return tail

                def make_mid(qb):
                    def mid():
                        proj_k(groups_of[qb])
                        proj_v(groups_of[qb])
                    return mid

                groups_of[0] = proj_block(0, load_x(0, fine=True))
                proj_k(groups_of[0])
                proj_q(groups_of[0])
                proj_v(groups_of[0])
                for qb in range(nq):
                    a_sb = atp.tile([128, npair, QB], BF16, tag="aT")
                    attn_block(
                        qb,
                        a_sb,
                        mid_emit=(make_mid(qb) if qb > 0 else None),
                        tail_emit=(make_tail(qb + 1) if qb + 1 < nq else None),
                        load_emit=(make_load(qb + 1) if qb + 1 < nq else None),
                        borrow_pp=(qb == nq - 1),
                    )
                    out_proj(qb, a_sb, tail=(qb == nq - 1))
            else:
                # unmasked attention reads ALL k/v: project everything first
                # (pp is idle during attention, so every block can borrow)
                for qb in range(nq):
                    groups = proj_block(qb, load_x(qb, fine=(qb == 0)))
                    proj_k(groups)
                    proj_q(groups)
                    proj_v(groups)
                for qb in range(nq):
                    a_sb = atp.tile([128, npair, QB], BF16, tag="aT")
                    attn_block(qb, a_sb, borrow_pp=True)
                    out_proj(qb, a_sb)

    return nc


# ---------------------------------------------------------------------------
# host side
# ---------------------------------------------------------------------------

_PROG_CACHE = {}
LAST_RESULT = None


def _get_program(mask_mode):
    if mask_mode not in _PROG_CACHE:
        _PROG_CACHE[mask_mode] = build_program(mask_mode)
    return _PROG_CACHE[mask_mode]


def _bf16(a):
    return np.ascontiguousarray(a).astype(BF16_NP)


def _pack_w(wT):
    """[D, m] -> [128, (D//128)*m] with row p holding chunks contiguously."""
    dch_, m = wT.shape[0] // 128, wT.shape[1]
    return np.ascontiguousarray(
        wT.reshape(dch_, 128, m).transpose(1, 0, 2).reshape(128, dch_ * m)
    )


def _pack_b(b):
    """[HD] -> [128, npair]: column c2 holds partitions of head-pair c2."""
    npair = HD // 128
    return np.ascontiguousarray(b.reshape(npair, 128).T).astype(np.float32)


def kernel(query, key_in, value, mask, w_q, b_q, w_k, b_k, w_v, b_v, w_out, b_out):
    from concourse.bass_utils import run_bass_kernel_spmd

    query = np.asarray(query, dtype=np.float32)
    key_in = np.asarray(key_in, dtype=np.float32)
    value = np.asarray(value, dtype=np.float32)
    mask = np.asarray(mask)
    w_q = np.asarray(w_q, dtype=np.float32)
    b_q = np.asarray(b_q, dtype=np.float32)
    w_k = np.asarray(w_k, dtype=np.float32)
    b_k = np.asarray(b_k, dtype=np.float32)
    w_v = np.asarray(w_v, dtype=np.float32)
    b_v = np.asarray(b_v, dtype=np.float32)
    w_out = np.asarray(w_out, dtype=np.float32)
    b_out = np.asarray(b_out, dtype=np.float32)

    scale = 1.0 / np.sqrt(np.float32(D_K))

    if (mask == 1).all():
        mode = "ones"
    elif all(
        np.array_equal(mask[b, 0], np.tril(np.ones((S, S), mask.dtype)))
        for b in range(mask.shape[0])
    ):
        mode = "causal"
    else:
        mode = "general"
    nc = _get_program(mode)

    wqT = _bf16(w_q.T * scale)   # [D, D] scaled
    wkT = _bf16(w_k.T)
    wvT = _bf16(w_v.T)
    woT = _bf16(w_out.T)
    bq_s = (b_q * scale).astype(np.float32)

    # per-batch transposed activations, shared by the 4 cores of a batch
    xqT = [_bf16(query[b].T) for b in range(B)]
    xkT = [_bf16(key_in[b].T) for b in range(B)]
    xvT = [_bf16(value[b].T) for b in range(B)]
    m01T = [_bf16(mask[b, 0].T) for b in range(B)] if mode == "general" else None

    in_maps = []
    for c in range(N_CORES):
        b = c // CORES_PER_BATCH
        hg = c % CORES_PER_BATCH
        hsl = slice(hg * HD, (hg + 1) * HD)
        im = {
            "xq": xqT[b],
            "xk": xkT[b],
            "xv": xvT[b],
            "wq": _pack_w(wqT[:, hsl]),
            "wk": _pack_w(wkT[:, hsl]),
            "wv": _pack_w(wvT[:, hsl]),
            "wo": _pack_w(woT[hsl, :]),
            "bq": _pack_b(bq_s[hsl]),
            "bk": _pack_b(b_k[hsl]),
        }
        if mode == "general":
            im["m01"] = m01T[b]
        in_maps.append(im)

    global LAST_RESULT
    try:
        res = run_bass_kernel_spmd(nc, in_maps, list(range(N_CORES)))
    except Exception:
        # transient NRT_EXEC_UNIT_UNRECOVERABLE wedges have been observed on
        # this fabric; a single retry has always cleared them
        import time as _time

        _time.sleep(3.0)
        res = run_bass_kernel_spmd(nc, in_maps, list(range(N_CORES)))
    LAST_RESULT = res

    b_eff = b_out + w_out @ b_v
    out = np.zeros((B, S, D_MODEL), dtype=np.float32)
    for c in range(N_CORES):
        out[c // CORES_PER_BATCH] += res.results[c]["out"].astype(np.float32)
    out += b_eff[None, None, :]
    return out
